# revision 11
# baseline (speedup 1.0000x reference)
"""Trainium2 Bass kernel for nn_CovarianceResidualError.

Computes, for errors [N, O] and graph_emb [N, D]:
    em   = errors - mean(errors, axis=0)
    a0   = (graph_emb - mean(graph_emb, axis=0))[:, :1]
    out  = -sum_o | sum_i em[i, o] * a0[i, 0] |

Identity used on device (exact in exact arithmetic):
    sum_i (e[i,o] - mean_e[o]) * (g[i] - mean_g)
      = sum_i e[i,o]*g[i]  -  mean_g * sum_i e[i,o]
(the mean_e term cancels because sum_i (g[i] - mean_g) == 0).

Sharding: data-parallel over N across 8 NeuronCores. Each core computes
partial P1[o] = sum_i e*g and P2[o] = sum_i e over its row shard (PE
matmul with a [g | 1] stationary weight pair per 128-row tile). The
O-length signed partials are combined on the host, which also computes
the scalar s = sum(g) (it already slices g per core; an on-device
partition fold would sit in the vector engine's in-order queue ahead of
the down-converts and stall them on a ~6us SBUF->SBUF DMA completion).
abs and the final sum happen strictly after the global sum.

Per-core pipeline (tuned against the NTFF trace):
  - e streams HBM->SBUF f32 on the sync HWDGE queue at ~420 GB/s in
    16 x 1MiB chunks; nothing else rides that queue.
  - each chunk's f32->bf16 down-convert is split across the vector AND
    scalar engines concurrently (vector is ~1.6x faster, so it takes
    5/8), keeping per-chunk convert latency ~0.9us against the 2.44us
    DMA pace; the PE consumes the bf16 sub-tiles at 1 cycle/row with
    f32 PSUM accumulation.
"""

import sys

if "/opt/trn_rl_repo" not in sys.path:
    sys.path.insert(0, "/opt/trn_rl_repo")

import numpy as np

import concourse.bacc as bacc
import concourse.mybir as mybir
import concourse.tile as tile
from concourse.bass_utils import run_bass_kernel_spmd

N, D, O = 131072, 128, 256
NCORES = 8
NLOC = N // NCORES          # 16384 rows per core
KP = 128                    # contraction (partition) dim per matmul
NT = NLOC // KP             # 128 sub-tiles per core
# DMA chunk sizes in 128KB sub-tiles. Uniform 1MiB chunks: every
# descriptor is a contiguous 8KB per partition. (A tapered tail of
# 256KB chunks measured faster on lucky runs, but its 2KB descriptors
# crawl when the chip's activity throttle lands on the stream tail —
# uniform 8KB descriptors are robust to throttle phasing.)
CHUNKS = [8] * 16
assert sum(CHUNKS) == NT
SUBMAX = max(CHUNKS)
EBUFS = 12                  # in-flight f32 e chunks (8KB/partition each)
BBUFS = 6                   # bf16 chunk buffers

_nc_cache = {}


def _vs_split(sub):
    """How many sub-tiles the vector engine converts (rest go to scalar).
    Vector sustains ~1.39 elem/ns/partition vs scalar's ~0.85."""
    return {16: 10, 8: 5, 4: 3, 2: 1, 1: 1}[sub]


def _build():
    f32 = mybir.dt.float32
    bf16 = mybir.dt.bfloat16
    nc = bacc.Bacc("TRN2", target_bir_lowering=False, debug=False,
                   num_devices=NCORES)
    e_ext = nc.dram_tensor("e", [NLOC, O], f32, kind="ExternalInput")
    g_ext = nc.dram_tensor("g", [NLOC, 1], f32, kind="ExternalInput")
    out_ext = nc.dram_tensor("out", [2 * O], f32, kind="ExternalOutput")

    # Interleaved row tiling: sub-tile t uses rows {k*NT + t, k=0..127}, so
    # partition k streams contiguous DRAM rows and the per-tile weight
    # column is a natural-layout column of g.
    e_r = e_ext.rearrange("(k t) o -> k t o", k=KP)          # [128, 128, 256]
    g_r = g_ext.rearrange("(p f) one -> p (f one)", p=KP)    # [128, 128]

    with tile.TileContext(nc) as tc:
        with (
            tc.tile_pool(name="const", bufs=1) as cpool,
            tc.tile_pool(name="io", bufs=EBUFS) as iopool,
            tc.tile_pool(name="bf", bufs=BBUFS) as bpool,
            tc.tile_pool(name="small", bufs=1) as spool,
            tc.tile_pool(name="psum", bufs=1, space="PSUM") as ppool,
        ):
            # first e chunk DMA goes first so the stream starts as early as
            # possible; nothing below gates it
            et0 = iopool.tile([KP, SUBMAX, O], f32, tag="et")
            nc.sync.dma_start(out=et0[:, 0:CHUNKS[0], :],
                              in_=e_r[:, 0:CHUNKS[0], :])

            # g loads go via gpsimd SWDGE so the sync HWDGE ring carries
            # nothing but the e stream
            g_nat = cpool.tile([KP, NT], f32)                 # g_nat[k,t] = g[k*128+t]
            nc.gpsimd.dma_start(out=g_nat[:], in_=g_r)
            # W[:, 2t] = g column for sub-tile t, W[:, 2t+1] = 1.0 (bf16 so
            # the PE streams 1 cycle/row instead of 4 for f32)
            w = cpool.tile([KP, 2 * NT], bf16)
            nc.vector.memset(w[:], 1.0)
            nc.vector.tensor_copy(out=w[:, 0:2 * NT:2], in_=g_nat[:])

            # main pass: psum[0,o] += sum_k g*e ; psum[1,o] += sum_k e
            psum_out = ppool.tile([2, O], f32)
            t0 = 0
            for b, sub in enumerate(CHUNKS):
                if b == 0:
                    et = et0
                else:
                    et = iopool.tile([KP, SUBMAX, O], f32, tag="et")
                    nc.sync.dma_start(out=et[:, 0:sub, :],
                                      in_=e_r[:, t0:t0 + sub, :])
                eb = bpool.tile([KP, SUBMAX, O], bf16, tag="eb")
                # split the f32->bf16 down-convert across both elementwise
                # engines so each chunk converts in <1us (DMA pace: 2.4us)
                nv = _vs_split(sub)
                nc.vector.tensor_copy(out=eb[:, 0:nv, :], in_=et[:, 0:nv, :])
                if nv < sub:
                    nc.scalar.copy(out=eb[:, nv:sub, :], in_=et[:, nv:sub, :])
                for j in range(sub):
                    t = t0 + j
                    nc.tensor.matmul(
                        psum_out[:],
                        lhsT=w[:, 2 * t:2 * t + 2],
                        rhs=eb[:, j, :],
                        start=(t == 0),
                        stop=(t == NT - 1),
                    )
                t0 += sub

            # pack [P1 | P2]; DMA cannot read PSUM, so bounce through SBUF
            part_sb = spool.tile([2, O], f32)
            nc.vector.tensor_copy(out=part_sb[:], in_=psum_out[:])
            nc.sync.dma_start(out=out_ext[0:2 * O], in_=part_sb[:])

    nc.compile()
    return nc


def _get_nc():
    if "nc" not in _nc_cache:
        _nc_cache["nc"] = _build()
    return _nc_cache["nc"]


def _make_in_maps(graph_emb, errors):
    errors = np.asarray(errors, dtype=np.float32)
    g = np.ascontiguousarray(np.asarray(graph_emb, dtype=np.float32)[:, 0:1])
    in_maps = []
    for c in range(NCORES):
        sl = slice(c * NLOC, (c + 1) * NLOC)
        in_maps.append({
            "e": np.ascontiguousarray(errors[sl]),
            "g": np.ascontiguousarray(g[sl]),
        })
    return in_maps


def _run(graph_emb, errors, **spmd_kwargs):
    nc = _get_nc()
    in_maps = _make_in_maps(graph_emb, errors)
    return run_bass_kernel_spmd(nc, in_maps, list(range(NCORES)), **spmd_kwargs)


def _combine_partials(results, s):
    """8-way sum of per-core [P1 | P2] partials, then
    col = P1 - (s/N)*P2 ; out = -sum |col|  (abs strictly after the
    global sum).  s = sum(g) is computed on the host."""
    acc = np.zeros(2 * O, dtype=np.float64)
    for r in results:
        acc += r["out"].astype(np.float64)
    p1, p2 = acc[0:O], acc[O:2 * O]
    col = p1 - (s / N) * p2
    return np.float32(-np.abs(col).sum())


def kernel(targets=None, out0=None, out1=None, graph_emb=None, errors=None,
           **_unused):
    res = _run(graph_emb, errors)
    s = float(np.asarray(graph_emb, dtype=np.float32)[:, 0].astype(np.float64).sum())
    val = _combine_partials(res.results, s)
    return np.asarray(val, dtype=np.float32).reshape(())
